# revision 1
# baseline (speedup 1.0000x reference)
"""Multi-head causal self-attention (B=2, S=2048, D=1024, H=16) on 8 TRN2 cores.

Sharding: head-parallel. Core c owns head-group c = heads {2c, 2c+1}
(= 128 of the 1024 qkv dims, both batches).

Per core:
  stage B: Q^T/K^T/V^T = (x @ W{q,k,v}[:, c-slice] + b)^T          [128, 4096]
  stage C: V^T -> V_aug [tok, 65] tiles (col 64 = ones, for the l-row trick)
  stage D: per (batch, head) pair: scores^T = K^T.T-tiles @ Q^T (PE),
           causal mask (DVE), exp (ACT, scale=1/8), ctx^T accum (PE) with
           the ones column producing l = sum(exp) in row 64.
  stage E: r = 1/l (DVE reciprocal_approx), broadcast via PE outer product,
           normalize ctx^T (DVE). AllGather ctx^T across the 8 cores.
  stage F: out^T[:, c-cols] = Wo[:, c-slice].T-tiles @ gathered ctx^T + bo.

Host: passes x pre-transposed, weight column slices; transposes y^T back.
"""

import sys

for p in ("/opt/trn_rl_repo", "/root/.axon_site/_ro/trn_rl_repo"):
    if p not in sys.path:
        sys.path.insert(0, p)

import numpy as np

import bass_rust
import concourse.bass as bass
import concourse.mybir as mybir
from concourse.bass_utils import run_bass_kernel_spmd
from concourse.masks import make_identity
from concourse.tile import TileContext

B, S, D = 2, 2048, 1024
H, DH = 16, 64
T = B * S              # 4096 tokens
NC = 8                 # cores
HG = D // NC           # 128 qkv dims per core (2 heads)
KT_D = D // 128        # 8 contraction tiles over d_model
INV_SCALE = 1.0 / float(np.sqrt(DH))  # 1/8
NEG = -1.0e9
F32 = mybir.dt.float32
F32R = mybir.dt.float32r
BF16 = mybir.dt.bfloat16
PO_DT = F32R  # BF16 would save ~33us but costs 8x accuracy


def _r(ap):
    return ap.bitcast(F32R)


def _split_waits(nc, max_waits=1):
    """This walrus build accepts one sync-wait per instruction; Tile sometimes
    emits more. Split extras into preceding NoOps on the same engine."""
    n = 0
    for f in nc.m.functions:
        for bb in f.blocks:
            out = []
            for inst in bb.instructions:
                si = getattr(inst, "sync_info", None)
                if si is not None and si.on_wait and len(si.on_wait) > max_waits:
                    waits = list(si.on_wait)
                    head, rest = waits[:-max_waits], waits[-max_waits:]
                    k = 0
                    while head:
                        chunk, head = head[:max_waits], head[max_waits:]
                        out.append(mybir.InstNoOp(
                            name=f"{inst.name}-wsplit-{k}", ins=[], outs=[],
                            engine=inst.engine,
                            sync_info=bass_rust.SyncInfo(on_wait=chunk, on_update=[]),
                        ))
                        k += 1
                    si.on_wait = rest
                    n += 1
                out.append(inst)
            bb.instructions = out
    return n


def build_module(repeat=1, stages="BCDEF", do_collective=True):
    nc = bass.Bass()

    xT = nc.dram_tensor("xT", [D, T], F32R, kind="ExternalInput")
    wq = nc.dram_tensor("wq", [D, HG], F32R, kind="ExternalInput")
    wk = nc.dram_tensor("wk", [D, HG], F32R, kind="ExternalInput")
    wv = nc.dram_tensor("wv", [D, HG], F32R, kind="ExternalInput")
    wo = nc.dram_tensor("wo", [HG, D], F32R, kind="ExternalInput")
    bq = nc.dram_tensor("bq", [HG, 1], F32, kind="ExternalInput")
    bk = nc.dram_tensor("bk", [HG, 1], F32, kind="ExternalInput")
    bv = nc.dram_tensor("bv", [HG, 1], F32, kind="ExternalInput")
    bo = nc.dram_tensor("bo", [HG, 1], F32, kind="ExternalInput")
    yT = nc.dram_tensor("yT", [HG, T], F32, kind="ExternalOutput")

    # per-batch partial out^T and reduce-scatter buffers: batch 0's RS
    # overlaps batch 1's attention; only batch 1's RS is an exposed tail
    po_b = [nc.dram_tensor(f"po{i}", [D, S], PO_DT) for i in range(B)]
    rs_b = [nc.dram_tensor(f"rs{i}", [HG, S], PO_DT) for i in range(B)]

    with TileContext(nc) as tc:
        with tc.tile_pool(name="persist", bufs=1) as pp:
            # weights as [128, kt, 128]
            w_sb = {}
            for name, dram in (("wq", wq), ("wk", wk), ("wv", wv)):
                t = pp.tile([128, KT_D, HG], F32R, name=f"{name}_sb", tag=f"{name}_sb")
                nc.sync.dma_start(out=t[:], in_=dram[:].rearrange("(kt p) n -> p kt n", p=128))
                w_sb[name] = t
            # wo: [HG rows of Wo, D out dims] -> [128, ot, 128]
            wo_sb = pp.tile([128, KT_D, 128], F32R, name="wo_sb", tag="wo_sb")
            nc.sync.dma_start(out=wo_sb[:], in_=wo[:].rearrange("p (ot n) -> p ot n", n=128))
            b_sb = {}
            for name, dram in (("bq", bq), ("bk", bk), ("bv", bv), ("bo", bo)):
                t = pp.tile([HG, 1], F32, name=f"{name}_sb", tag=f"{name}_sb")
                nc.sync.dma_start(out=t[:], in_=dram[:])
                b_sb[name] = t

            # identity built in f32 (gpsimd memset can't write f32r),
            # then DVE-copied (rounds) into the f32r tile matmul needs
            ident_f = pp.tile([128, 128], F32, name="ident_f", tag="ident_f")
            make_identity(nc, ident_f[:])
            ident = pp.tile([128, 128], F32R, name="ident", tag="ident")
            nc.vector.tensor_copy(ident[:], ident_f[:])
            # additive causal mask for a diagonal 128x128 tile in scores^T
            # layout: tri[r, c] = 0 where r <= c (k <= q), else -1e9
            tri01 = pp.tile([128, 128], F32, name="tri01", tag="tri01")
            nc.gpsimd.memset(tri01[:], 1.0)
            # keep 1 where c - r >= 0 (k <= q); 0 strictly below the diagonal
            nc.gpsimd.affine_select(
                out=tri01[:], in_=tri01[:],
                compare_op=mybir.AluOpType.is_ge, fill=0.0,
                base=0, pattern=[[1, 128]], channel_multiplier=-1,
            )
            # ones on partition row 64 (same base partition as the l row)
            ones_sb = pp.tile([65, 128], F32, name="ones_sb", tag="ones_sb")
            nc.vector.memset(ones_sb[:], 1.0)
            ones128 = pp.tile([128, 64], F32, name="ones128", tag="ones128")
            nc.vector.memset(ones128[:], 1.0)
            ones_r = pp.tile([65, 128], F32R, name="ones_r", tag="ones_r")
            nc.vector.tensor_copy(ones_r[:], ones_sb[:])

            # per-batch Q^T/K^T/V^T so batch 1's projection overlaps batch 0's
            # attention
            qkvT = {}
            for name in ("qT", "kT", "vT"):
                qkvT[name] = [pp.tile([128, S], F32R, name=f"{name}{b}", tag=f"{name}{b}")
                              for b in range(B)]

            vaug = pp.tile([128, B * 2, S // 128, DH + 1], F32R, name="vaug", tag="vaug")
            nc.vector.tensor_copy(vaug[:, :, :, DH:DH + 1], ones128[:, :])
            # [65 used partitions, pair, q]; row 64 = l
            ctxu = pp.tile([128, B * 2, S], F32, name="ctxu", tag="ctxu")
            ctxn = pp.tile([128, T], F32R, name="ctxn", tag="ctxn")

            for _rep in range(repeat):
                if "B" not in stages:
                    break
                # ---------------- stage B: QKV projections (both batches) ----------------
                with (
                    tc.tile_pool(name="xt_pool", bufs=4) as xt_pool,
                    tc.tile_pool(name="psB", bufs=1, space="PSUM") as psB_pool,
                    tc.tile_pool(name="psT", bufs=2, space="PSUM") as psT_pool,
                ):
                    for b in range(B):
                        for tq in range(2):
                            t0 = tq * 1024
                            ps = [psB_pool.tile([128, 512], F32, name=f"psB{i}",
                                                tag=f"psB{i}") for i in range(6)]
                            for kt in range(KT_D):
                                xt = xt_pool.tile([128, 1024], F32R, name="xt", tag="xt")
                                nc.sync.dma_start(
                                    out=xt[:],
                                    in_=xT[kt * 128:(kt + 1) * 128,
                                           b * S + t0: b * S + t0 + 1024])
                                for pi, wname in enumerate(("wq", "wk", "wv")):
                                    for nch in range(2):
                                        nc.tensor.matmul(
                                            ps[pi * 2 + nch][:],
                                            w_sb[wname][:, kt, :],
                                            xt[:, nch * 512:(nch + 1) * 512],
                                            start=(kt == 0), stop=(kt == KT_D - 1),
                                        )
                            for pi, (dname, bname) in enumerate(
                                    (("qT", "bq"), ("kT", "bk"), ("vT", "bv"))):
                                for nch in range(2):
                                    nc.vector.tensor_scalar_add(
                                        out=qkvT[dname][b][:, t0 + nch * 512:
                                                           t0 + (nch + 1) * 512],
                                        in0=ps[pi * 2 + nch][:],
                                        scalar1=b_sb[bname][:, 0:1],
                                    )
                        if "C" not in stages:
                            continue
                        # ---- stage C: V^T -> V_aug for batch b ----
                        for h in range(2):
                            pr = b * 2 + h
                            for g in range(2):  # groups of 8 ktiles
                                pst = psT_pool.tile([128, 512], F32R, name="pst", tag="pst")
                                for j in range(8):
                                    kt = g * 8 + j
                                    nc.tensor.transpose(
                                        out=pst[:, j * DH:(j + 1) * DH],
                                        in_=qkvT["vT"][b][h * DH:(h + 1) * DH,
                                                          kt * 128:(kt + 1) * 128],
                                        identity=ident[h * DH:(h + 1) * DH,
                                                       h * DH:(h + 1) * DH],
                                    )
                                nc.vector.tensor_copy(
                                    vaug[:, pr, g * 8:(g + 1) * 8, 0:DH],
                                    pst[:],
                                )
                if "D" not in stages:
                    continue
                # ------- stages D-G, pipelined per (batch, q-chunk) -------
                with (
                    tc.tile_pool(name="psS", bufs=2, space="PSUM") as psS_pool,
                    tc.tile_pool(name="psC", bufs=2, space="PSUM") as psC_pool,
                    tc.tile_pool(name="misc", bufs=2, space="PSUM") as misc_pool,
                    tc.tile_pool(name="exp_pool", bufs=4) as exp_pool,
                    tc.tile_pool(name="rpool", bufs=1) as rpool,
                    tc.tile_pool(name="fo_pool", bufs=4) as fo_pool,
                    tc.tile_pool(name="gy_pool", bufs=1) as gy_pool,
                ):
                    for b in range(B):
                        r_ts = []
                        for h in range(2):
                            # ---- stage D: attention for (batch b, head h) ----
                            pr = b * 2 + h
                            qT_h = qkvT["qT"][b][h * DH:(h + 1) * DH, :]
                            kT_h = qkvT["kT"][b][h * DH:(h + 1) * DH, :]
                            for qc in range(S // 512):
                                q0 = qc * 512
                                n_kt = q0 // 128 + 4
                                ps_ctx = psC_pool.tile([128, 512], F32, name="ps_ctx",
                                                       tag="ps_ctx")
                                for kg in range(n_kt // 2):
                                    ka, kb = 2 * kg, 2 * kg + 1
                                    offa = max(0, ka * 128 - q0)
                                    offb = max(0, kb * 128 - q0)
                                    ps_s = psS_pool.tile([128, 1024], F32, name="ps_s",
                                                         tag="ps_s")
                                    nc.tensor.matmul(
                                        ps_s[:, offa:512],
                                        kT_h[:, ka * 128:(ka + 1) * 128],
                                        qT_h[:, q0 + offa:q0 + 512],
                                        start=True, stop=True,
                                    )
                                    nc.tensor.matmul(
                                        ps_s[:, 512 + offb:1024],
                                        kT_h[:, kb * 128:(kb + 1) * 128],
                                        qT_h[:, q0 + offb:q0 + 512],
                                        start=True, stop=True,
                                    )
                                    ex = exp_pool.tile([128, 1024], F32R, name="ex", tag="ex")
                                    # one exp over both halves; the gap
                                    # [512:512+offb) holds stale-but-finite data
                                    # that the ctx matmuls never read.
                                    nc.scalar.activation(
                                        out=ex[:, offa:1024], in_=ps_s[:, offa:1024],
                                        func=mybir.ActivationFunctionType.Exp,
                                        scale=INV_SCALE,
                                    )
                                    # causal mask: multiplicative 0/1 on the
                                    # diagonal tiles, applied AFTER exp so DVE
                                    # stays off the PE->ACT critical path
                                    if ka * 128 >= q0:
                                        nc.vector.tensor_mul(
                                            out=ex[:, offa:offa + 128],
                                            in0=ex[:, offa:offa + 128],
                                            in1=tri01[:],
                                        )
                                    if kb * 128 >= q0:
                                        nc.vector.tensor_mul(
                                            out=ex[:, 512 + offb:512 + offb + 128],
                                            in0=ex[:, 512 + offb:512 + offb + 128],
                                            in1=tri01[:],
                                        )
                                    nc.tensor.matmul(
                                        ps_ctx[0:DH + 1, offa:512],
                                        vaug[:, pr, ka, :],
                                        ex[:, offa:512],
                                        start=(ka == 0), stop=False,
                                        skip_group_check=True,
                                    )
                                    nc.tensor.matmul(
                                        ps_ctx[0:DH + 1, offb:512],
                                        vaug[:, pr, kb, :],
                                        ex[:, 512 + offb:1024],
                                        start=False, stop=(kb == n_kt - 1),
                                        skip_group_check=True,
                                    )
                                nc.vector.tensor_copy(
                                    ctxu[0:DH + 1, pr, q0:q0 + 512],
                                    ps_ctx[0:DH + 1, :],
                                )
                            if "E" not in stages:
                                continue
                            # ---- stage E: r = 1/l = exp(-ln(l)) for this head ----
                            ln_f = rpool.tile([65, S], F32, name="ln_f", tag="ln_f")
                            nc.scalar.activation(
                                out=ln_f[64:65, :], in_=ctxu[64:65, pr, :],
                                func=mybir.ActivationFunctionType.Ln)
                            r_t = rpool.tile([65, S], F32R, name=f"r_t{h}", tag=f"r_t{h}")
                            nc.scalar.activation(
                                out=r_t[64:65, :], in_=ln_f[64:65, :],
                                func=mybir.ActivationFunctionType.Exp, scale=-1.0)
                            r_ts.append(r_t)
                        # ---- per q-chunk: normalize + partial out^T ----
                        if "F" not in stages or "E" not in stages:
                            continue
                        for qc in range(S // 512):
                            q0 = qc * 512
                            for h in range(2):
                                pr = b * 2 + h
                                bc = misc_pool.tile([128, 512], F32, name="bc", tag="efps")
                                nc.tensor.matmul(
                                    bc[0:DH, :],
                                    ones_r[64:65, 0:DH],
                                    r_ts[h][64:65, q0:q0 + 512],
                                    start=True, stop=True,
                                )
                                nc.vector.tensor_mul(
                                    out=ctxn[h * DH:(h + 1) * DH,
                                             b * S + q0: b * S + q0 + 512],
                                    in0=ctxu[0:DH, pr, q0:q0 + 512],
                                    in1=bc[0:DH, :],
                                )
                            for ot in range(KT_D):
                                ps_o = misc_pool.tile([128, 512], F32, name="ps_o",
                                                      tag="efps")
                                nc.tensor.matmul(
                                    ps_o[:],
                                    wo_sb[:, ot, :],
                                    ctxn[:, b * S + q0: b * S + q0 + 512],
                                    start=True, stop=True,
                                )
                                pot = fo_pool.tile([128, 512], PO_DT, name="pot", tag="pot")
                                if ot % 2 == 0:
                                    nc.vector.tensor_copy(pot[:], ps_o[:])
                                else:
                                    nc.scalar.activation(
                                        out=pot[:], in_=ps_o[:],
                                        func=mybir.ActivationFunctionType.Copy)
                                nc.sync.dma_start(
                                    out=po_b[b][ot * 128:(ot + 1) * 128, q0:q0 + 512],
                                    in_=pot[:])
                        if not do_collective:
                            continue
                        nc.gpsimd.collective_compute(
                            "ReduceScatter",
                            mybir.AluOpType.add,
                            ins=[po_b[b][:]],
                            outs=[rs_b[b][:]],
                            replica_groups=[list(range(NC))],
                        )
                        # ---- stage G: + bo, store batch b ----
                        yt_in = gy_pool.tile([128, S], PO_DT, name="yt_in", tag="yt_in")
                        nc.sync.dma_start(out=yt_in[:], in_=rs_b[b][:])
                        yo = gy_pool.tile([128, S], F32, name="yo", tag="yo")
                        nc.vector.tensor_scalar_add(
                            out=yo[:], in0=yt_in[:], scalar1=b_sb["bo"][:, 0:1])
                        nc.sync.dma_start(out=yT[:, b * S:(b + 1) * S], in_=yo[:])

    _split_waits(nc)
    return nc


def kernel(x, mask, Wq, bq, Wk, bk, Wv, bv, Wo, bo, trace=False, repeat=1, _in_maps_only=False):
    x = np.asarray(x, dtype=np.float32).reshape(T, D)
    xT = np.ascontiguousarray(x.T)
    in_maps = []
    for c in range(NC):
        sl = slice(c * HG, (c + 1) * HG)
        in_maps.append({
            "xT": xT,
            "wq": np.ascontiguousarray(np.asarray(Wq, np.float32)[:, sl]),
            "wk": np.ascontiguousarray(np.asarray(Wk, np.float32)[:, sl]),
            "wv": np.ascontiguousarray(np.asarray(Wv, np.float32)[:, sl]),
            "wo": np.ascontiguousarray(np.asarray(Wo, np.float32)[sl, :]),
            "bq": np.ascontiguousarray(np.asarray(bq, np.float32)[sl].reshape(HG, 1)),
            "bk": np.ascontiguousarray(np.asarray(bk, np.float32)[sl].reshape(HG, 1)),
            "bv": np.ascontiguousarray(np.asarray(bv, np.float32)[sl].reshape(HG, 1)),
            "bo": np.ascontiguousarray(np.asarray(bo, np.float32)[sl].reshape(HG, 1)),
        })
    if _in_maps_only:
        return in_maps
    nc = build_module(repeat=repeat)
    res = run_bass_kernel_spmd(nc, in_maps, core_ids=list(range(NC)), trace=trace)
    out = np.empty((T, D), dtype=np.float32)
    for c in range(NC):
        out[:, c * HG:(c + 1) * HG] = res.results[c]["yT"].T
    if trace:
        kernel.last_results = res
    return out.reshape(B, S, D)



# revision 4
# speedup vs baseline: 1.6457x; 1.6457x over previous
"""Multi-head causal self-attention (B=2, S=2048, D=1024, H=16) on 8 TRN2 cores.

Sharding: head-parallel. Core c owns head-group c = heads {2c, 2c+1}
(= 128 of the 1024 qkv dims, both batches) and out-dims [128c, 128c+128).

Per core:
  stage B: Q^T/K^T/V^T = (x @ W{q,k,v}[:, c-slice] + b)^T            [128, 4096]
           (bf16 operands, f32 PSUM accumulation)
  stage C: V^T -> V_aug [tok, 65] tiles (col 64 = ones, for the l-row trick)
  stage D: per (batch, q-chunk, k-tile): scores^T for BOTH heads with one
           [128,1024] PSUM tile — h0 on PE rows 0-63, h1 on rows 64-127
           (different row groups -> the two matmuls run concurrently).
           One exp (ACT, scale=1/8, bf16) covers both heads; causal mask
           (DVE) on diagonal tiles; ctx^T accumulation per head with the
           ones column producing l = sum(exp) in row 64.
  stage E: per (batch, q-chunk): r = 1/l (Ln+Exp), broadcast via PE outer
           product, normalize ctx^T to bf16, DMA out, AllGather the
           [128, 512] bf16 shard -> [1024, 512] full normalized ctx^T.
           Early chunks' AllGathers overlap later attention.
  stage F: out^T[c-slice rows, :] = Wo[:, c-slice].T-tiles @ gathered ctx^T
           (full 1024 contraction, 128 out dims per core) + bo.

Host: passes x pre-transposed in bf16, weight column slices in bf16;
transposes y^T back.
"""

import sys

for p in ("/opt/trn_rl_repo", "/root/.axon_site/_ro/trn_rl_repo"):
    if p not in sys.path:
        sys.path.insert(0, p)

import numpy as np
import ml_dtypes

import bass_rust
import concourse.bass as bass
import concourse.mybir as mybir
from concourse.bass_utils import run_bass_kernel_spmd
from concourse.masks import make_identity
from concourse.tile import TileContext

B, S, D = 2, 2048, 1024
H, DH = 16, 64
T = B * S              # 4096 tokens
NC = 8                 # cores
HG = D // NC           # 128 qkv dims per core (2 heads)
KT_D = D // 128        # 8 contraction tiles over d_model
NQC = S // 512         # 4 q-chunks per batch
INV_SCALE = 1.0 / float(np.sqrt(DH))  # 1/8
F32 = mybir.dt.float32
F32R = mybir.dt.float32r
BF16 = mybir.dt.bfloat16
BFNP = ml_dtypes.bfloat16


def _split_waits(nc, max_waits=1):
    """This walrus build accepts one sync-wait per instruction; Tile sometimes
    emits more. Split extras into preceding NoOps on the same engine."""
    n = 0
    for f in nc.m.functions:
        for bb in f.blocks:
            out = []
            for inst in bb.instructions:
                si = getattr(inst, "sync_info", None)
                if si is not None and si.on_wait and len(si.on_wait) > max_waits:
                    waits = list(si.on_wait)
                    head, rest = waits[:-max_waits], waits[-max_waits:]
                    k = 0
                    while head:
                        chunk, head = head[:max_waits], head[max_waits:]
                        out.append(mybir.InstNoOp(
                            name=f"{inst.name}-wsplit-{k}", ins=[], outs=[],
                            engine=inst.engine,
                            sync_info=bass_rust.SyncInfo(on_wait=chunk, on_update=[]),
                        ))
                        k += 1
                    si.on_wait = rest
                    n += 1
                out.append(inst)
            bb.instructions = out
    return n


def build_module(repeat=1, stages="BCDEF", do_collective=True):
    nc = bass.Bass()

    xT = nc.dram_tensor("xT", [D, T], BF16, kind="ExternalInput")
    wq = nc.dram_tensor("wq", [D, HG], BF16, kind="ExternalInput")
    wk = nc.dram_tensor("wk", [D, HG], BF16, kind="ExternalInput")
    wv = nc.dram_tensor("wv", [D, HG], BF16, kind="ExternalInput")
    wo = nc.dram_tensor("wo", [D, HG], BF16, kind="ExternalInput")
    bq = nc.dram_tensor("bq", [HG, 1], F32, kind="ExternalInput")
    bk = nc.dram_tensor("bk", [HG, 1], F32, kind="ExternalInput")
    bv = nc.dram_tensor("bv", [HG, 1], F32, kind="ExternalInput")
    bo = nc.dram_tensor("bo", [HG, 1], F32, kind="ExternalInput")
    yT = nc.dram_tensor("yT", [HG, T], F32, kind="ExternalOutput")

    # AllGather staging: per (batch, q-chunk) normalized ctx^T shard and
    # gathered full ctx^T
    agi = [[nc.dram_tensor(f"agi{b}_{qc}", [HG, 512], BF16)
            for qc in range(NQC)] for b in range(B)]
    ago = [[nc.dram_tensor(f"ago{b}_{qc}", [D, 512], BF16, addr_space="Shared")
            for qc in range(NQC)] for b in range(B)]

    with TileContext(nc) as tc:
        with tc.tile_pool(name="persist", bufs=1) as pp:
            # weights as [128, kt, 128] (contraction chunk is the partition dim)
            w_sb = {}
            for name, dram in (("wq", wq), ("wk", wk), ("wv", wv), ("wo", wo)):
                t = pp.tile([128, KT_D, HG], BF16, name=f"{name}_sb", tag=f"{name}_sb")
                nc.sync.dma_start(out=t[:], in_=dram[:].rearrange("(kt p) n -> p kt n", p=128))
                w_sb[name] = t
            b_sb = {}
            for name, dram in (("bq", bq), ("bk", bk), ("bv", bv), ("bo", bo)):
                t = pp.tile([HG, 1], F32, name=f"{name}_sb", tag=f"{name}_sb")
                nc.sync.dma_start(out=t[:], in_=dram[:])
                b_sb[name] = t

            # identity built in f32 (gpsimd memset can't write bf16 reliably),
            # then DVE-copied (rounds) into the bf16 tile matmul needs
            ident_f = pp.tile([128, 128], F32, name="ident_f", tag="ident_f")
            make_identity(nc, ident_f[:])
            ident = pp.tile([128, 128], BF16, name="ident", tag="ident")
            nc.vector.tensor_copy(ident[:], ident_f[:])
            # multiplicative causal mask for a diagonal 128x128 tile in
            # scores^T: tri01[r, c] = 1 where r <= c (k <= q), else 0
            tri_f = pp.tile([128, 128], F32, name="tri_f", tag="tri_f")
            nc.gpsimd.memset(tri_f[:], 1.0)
            nc.gpsimd.affine_select(
                out=tri_f[:], in_=tri_f[:],
                compare_op=mybir.AluOpType.is_ge, fill=0.0,
                base=0, pattern=[[1, 128]], channel_multiplier=-1,
            )
            tri01 = pp.tile([128, 128], BF16, name="tri01", tag="tri01")
            nc.vector.tensor_copy(tri01[:], tri_f[:])
            # ones row for the r-broadcast outer product (f32r, full speed)
            ones_r = pp.tile([65, 128], F32R, name="ones_r", tag="ones_r")
            of = pp.tile([65, 128], F32, name="of", tag="of")
            nc.vector.memset(of[:], 1.0)
            nc.vector.tensor_copy(ones_r[:], of[:])
            ones128 = pp.tile([128, 64], F32, name="ones128", tag="ones128")
            nc.vector.memset(ones128[:], 1.0)

            # per-batch Q^T/K^T/V^T so batch 1's projection overlaps batch 0's
            # attention
            qkvT = {}
            for name in ("qT", "kT", "vT"):
                qkvT[name] = [pp.tile([128, S], BF16, name=f"{name}{b}", tag=f"{name}{b}")
                              for b in range(B)]

            vaug = pp.tile([128, B * 2, S // 128, DH + 1], BF16, name="vaug", tag="vaug")
            nc.vector.tensor_copy(vaug[:, :, :, DH:DH + 1], ones128[:, :])
            # [65 used partitions, pair, q]; row 64 = l
            ctxu = pp.tile([128, B * 2, S], F32, name="ctxu", tag="ctxu")

            for _rep in range(repeat):
                if "B" not in stages:
                    break
                # ---------------- stage B: QKV projections (both batches) ----------------
                with (
                    tc.tile_pool(name="xt_pool", bufs=4) as xt_pool,
                    tc.tile_pool(name="psB", bufs=1, space="PSUM") as psB_pool,
                    tc.tile_pool(name="psT", bufs=2, space="PSUM") as psT_pool,
                ):
                    for b in range(B):
                        for tq in range(2):
                            t0 = tq * 1024
                            ps = [psB_pool.tile([128, 512], F32, name=f"psB{i}",
                                                tag=f"psB{i}") for i in range(6)]
                            for kt in range(KT_D):
                                xt = xt_pool.tile([128, 1024], BF16, name="xt", tag="xt")
                                nc.sync.dma_start(
                                    out=xt[:],
                                    in_=xT[kt * 128:(kt + 1) * 128,
                                           b * S + t0: b * S + t0 + 1024])
                                for pi, wname in enumerate(("wq", "wk", "wv")):
                                    for nch in range(2):
                                        nc.tensor.matmul(
                                            ps[pi * 2 + nch][:],
                                            w_sb[wname][:, kt, :],
                                            xt[:, nch * 512:(nch + 1) * 512],
                                            start=(kt == 0), stop=(kt == KT_D - 1),
                                        )
                            for pi, (dname, bname) in enumerate(
                                    (("qT", "bq"), ("kT", "bk"), ("vT", "bv"))):
                                for nch in range(2):
                                    nc.vector.tensor_scalar_add(
                                        out=qkvT[dname][b][:, t0 + nch * 512:
                                                           t0 + (nch + 1) * 512],
                                        in0=ps[pi * 2 + nch][:],
                                        scalar1=b_sb[bname][:, 0:1],
                                    )
                        if "C" not in stages:
                            continue
                        # ---- stage C: V^T -> V_aug for batch b ----
                        for h in range(2):
                            pr = b * 2 + h
                            for g in range(2):  # groups of 8 ktiles
                                pst = psT_pool.tile([128, 512], BF16, name="pst", tag="pst")
                                for j in range(8):
                                    kt = g * 8 + j
                                    nc.tensor.transpose(
                                        out=pst[:, j * DH:(j + 1) * DH],
                                        in_=qkvT["vT"][b][h * DH:(h + 1) * DH,
                                                          kt * 128:(kt + 1) * 128],
                                        identity=ident[h * DH:(h + 1) * DH,
                                                       h * DH:(h + 1) * DH],
                                    )
                                nc.vector.tensor_copy(
                                    vaug[:, pr, g * 8:(g + 1) * 8, 0:DH],
                                    pst[:],
                                )
                if "D" not in stages:
                    continue
                # ------- stages D-F, pipelined per (batch, q-chunk) -------
                with (
                    tc.tile_pool(name="psS", bufs=2, space="PSUM") as psS_pool,
                    tc.tile_pool(name="psC", bufs=2, space="PSUM") as psC_pool,
                    tc.tile_pool(name="misc", bufs=2, space="PSUM") as misc_pool,
                    tc.tile_pool(name="exp_pool", bufs=4) as exp_pool,
                    tc.tile_pool(name="rpool", bufs=2) as rpool,
                    tc.tile_pool(name="cn_pool", bufs=3) as cn_pool,
                    tc.tile_pool(name="gx_pool", bufs=2) as gx_pool,
                    tc.tile_pool(name="yo_pool", bufs=2) as yo_pool,
                ):
                    for b in range(B):
                        pr0, pr1 = b * 2, b * 2 + 1
                        qT0 = qkvT["qT"][b][0:DH, :]
                        kT0 = qkvT["kT"][b][0:DH, :]
                        qT1 = qkvT["qT"][b][DH:2 * DH, :]
                        kT1 = qkvT["kT"][b][DH:2 * DH, :]
                        for qc in range(NQC):
                            q0 = qc * 512
                            n_kt = q0 // 128 + 4
                            # ---- stage D: both heads interleaved per k-tile ----
                            ps_c0 = psC_pool.tile([128, 512], F32, name="ps_c0",
                                                  tag="ps_ctx")
                            ps_c1 = psC_pool.tile([128, 512], F32, name="ps_c1",
                                                  tag="ps_ctx")
                            for kt in range(n_kt):
                                off = max(0, kt * 128 - q0)
                                ps_s = psS_pool.tile([128, 1024], F32, name="ps_s",
                                                     tag="ps_s")
                                # h0 on PE rows 0-63, h1 on rows 64-127: the two
                                # matmuls occupy different row groups and run
                                # concurrently
                                nc.tensor.matmul(
                                    ps_s[:, off:512],
                                    kT0[:, kt * 128:(kt + 1) * 128],
                                    qT0[:, q0 + off:q0 + 512],
                                    start=True, stop=True,
                                )
                                nc.tensor.matmul(
                                    ps_s[:, 512 + off:1024],
                                    kT1[:, kt * 128:(kt + 1) * 128],
                                    qT1[:, q0 + off:q0 + 512],
                                    start=True, stop=True,
                                )
                                ex = exp_pool.tile([128, 1024], BF16, name="ex", tag="ex")
                                # one exp over both heads' halves; the gap
                                # [512:512+off) holds stale-but-finite data
                                # that the ctx matmuls never read.
                                nc.scalar.activation(
                                    out=ex[:, off:1024], in_=ps_s[:, off:1024],
                                    func=mybir.ActivationFunctionType.Exp,
                                    scale=INV_SCALE,
                                )
                                if kt * 128 >= q0:
                                    # diagonal tile: multiplicative causal mask,
                                    # applied AFTER exp so DVE stays off the
                                    # PE->ACT critical path
                                    nc.vector.tensor_mul(
                                        out=ex[:, off:off + 128],
                                        in0=ex[:, off:off + 128],
                                        in1=tri01[:],
                                    )
                                    nc.vector.tensor_mul(
                                        out=ex[:, 512 + off:512 + off + 128],
                                        in0=ex[:, 512 + off:512 + off + 128],
                                        in1=tri01[:],
                                    )
                                nc.tensor.matmul(
                                    ps_c0[0:DH + 1, off:512],
                                    vaug[:, pr0, kt, :],
                                    ex[:, off:512],
                                    start=(kt == 0), stop=(kt == n_kt - 1),
                                    skip_group_check=True,
                                )
                                nc.tensor.matmul(
                                    ps_c1[0:DH + 1, off:512],
                                    vaug[:, pr1, kt, :],
                                    ex[:, 512 + off:1024],
                                    start=(kt == 0), stop=(kt == n_kt - 1),
                                    skip_group_check=True,
                                )
                            nc.vector.tensor_copy(
                                ctxu[0:DH + 1, pr0, q0:q0 + 512], ps_c0[0:DH + 1, :])
                            nc.vector.tensor_copy(
                                ctxu[0:DH + 1, pr1, q0:q0 + 512], ps_c1[0:DH + 1, :])
                            if "E" not in stages:
                                continue
                            # ---- stage E: r = 1/l = exp(-ln(l)), both heads ----
                            ln_f = rpool.tile([65, 2, 512], F32, name="ln_f", tag="ln_f")
                            nc.scalar.activation(
                                out=ln_f[64:65, :, :],
                                in_=ctxu[64:65, pr0:pr0 + 2, q0:q0 + 512],
                                func=mybir.ActivationFunctionType.Ln)
                            r_t = rpool.tile([65, 2, 512], F32R, name="r_t", tag="r_t")
                            nc.scalar.activation(
                                out=r_t[64:65, :, :], in_=ln_f[64:65, :, :],
                                func=mybir.ActivationFunctionType.Exp, scale=-1.0)
                            # normalize ctx^T to bf16 and AllGather this q-chunk
                            cn = cn_pool.tile([128, 512], BF16, name="cn", tag="cn")
                            for h in range(2):
                                bcst = misc_pool.tile([128, 512], F32, name="bc",
                                                      tag="efps")
                                nc.tensor.matmul(
                                    bcst[0:DH, :],
                                    ones_r[64:65, 0:DH],
                                    r_t[64:65, h, :],
                                    start=True, stop=True,
                                )
                                nc.vector.tensor_mul(
                                    out=cn[h * DH:(h + 1) * DH, :],
                                    in0=ctxu[0:DH, b * 2 + h, q0:q0 + 512],
                                    in1=bcst[0:DH, :],
                                )
                            nc.sync.dma_start(out=agi[b][qc][:], in_=cn[:])
                            if do_collective:
                                nc.gpsimd.collective_compute(
                                    "AllGather",
                                    mybir.AluOpType.bypass,
                                    ins=[agi[b][qc][:]],
                                    outs=[ago[b][qc][:]],
                                    replica_groups=[list(range(NC))],
                                )
                    # ---- stage F: out^T = Wo_cols^T-tiles @ gathered ctx^T + bo ----
                    if "F" in stages and "E" in stages and do_collective:
                        for b in range(B):
                            for qc in range(NQC):
                                q0 = qc * 512
                                gx = gx_pool.tile([128, KT_D, 512], BF16,
                                                  name="gx", tag="gx")
                                nc.sync.dma_start(
                                    out=gx[:],
                                    in_=ago[b][qc][:].rearrange(
                                        "(kt p) q -> p kt q", p=128))
                                ps_o = misc_pool.tile([128, 512], F32, name="ps_o",
                                                      tag="efps")
                                for kt in range(KT_D):
                                    nc.tensor.matmul(
                                        ps_o[:],
                                        w_sb["wo"][:, kt, :],
                                        gx[:, kt, :],
                                        start=(kt == 0), stop=(kt == KT_D - 1),
                                    )
                                yo = yo_pool.tile([128, 512], F32, name="yo", tag="yo")
                                nc.vector.tensor_scalar_add(
                                    out=yo[:], in0=ps_o[:],
                                    scalar1=b_sb["bo"][:, 0:1])
                                nc.sync.dma_start(
                                    out=yT[:, b * S + q0:b * S + q0 + 512],
                                    in_=yo[:])

    _split_waits(nc)
    return nc


def kernel(x, mask, Wq, bq, Wk, bk, Wv, bv, Wo, bo, trace=False, repeat=1, _in_maps_only=False):
    x = np.asarray(x, dtype=np.float32).reshape(T, D)
    xT = np.ascontiguousarray(x.T.astype(BFNP))
    in_maps = []
    for c in range(NC):
        sl = slice(c * HG, (c + 1) * HG)
        in_maps.append({
            "xT": xT,
            "wq": np.ascontiguousarray(np.asarray(Wq, np.float32)[:, sl].astype(BFNP)),
            "wk": np.ascontiguousarray(np.asarray(Wk, np.float32)[:, sl].astype(BFNP)),
            "wv": np.ascontiguousarray(np.asarray(Wv, np.float32)[:, sl].astype(BFNP)),
            "wo": np.ascontiguousarray(np.asarray(Wo, np.float32)[:, sl].astype(BFNP)),
            "bq": np.ascontiguousarray(np.asarray(bq, np.float32)[sl].reshape(HG, 1)),
            "bk": np.ascontiguousarray(np.asarray(bk, np.float32)[sl].reshape(HG, 1)),
            "bv": np.ascontiguousarray(np.asarray(bv, np.float32)[sl].reshape(HG, 1)),
            "bo": np.ascontiguousarray(np.asarray(bo, np.float32)[sl].reshape(HG, 1)),
        })
    if _in_maps_only:
        return in_maps
    nc = build_module(repeat=repeat)
    res = run_bass_kernel_spmd(nc, in_maps, core_ids=list(range(NC)), trace=trace)
    out = np.empty((T, D), dtype=np.float32)
    for c in range(NC):
        out[:, c * HG:(c + 1) * HG] = res.results[c]["yT"].T
    if trace:
        kernel.last_results = res
    return out.reshape(B, S, D)


# revision 8
# speedup vs baseline: 1.9002x; 1.1546x over previous
"""Multi-head causal self-attention (B=2, S=2048, D=1024, H=16) on 8 TRN2 cores.

Sharding: head-parallel attention + token-parallel output projection.
Core c owns head-group c = heads {2c, 2c+1} (= 128 of the 1024 qkv dims,
both batches) for stages B-E, then tokens [256c, 256c+256) of each batch
for stage F (Wo replicated).

Per core:
  stage B: Q^T/K^T/V^T = (x @ W{q,k,v}[:, c-slice] + b)^T            [128, 4096]
           (bf16 operands, f32 PSUM accumulation, 512-token PSUM chunks
           double-buffered)
  stage C: V^T -> V_aug [tok, 65] tiles (col 64 = ones, for the l-row trick)
  stage D: per (batch, q-chunk, k-tile): scores^T for BOTH heads with one
           [128,1024] PSUM tile — h0 on PE rows 0-63, h1 on rows 64-127
           (different row groups -> the two matmuls run concurrently).
           One exp (ACT, scale=1/8, bf16) covers both heads; causal mask
           (DVE) on diagonal tiles; ctx^T accumulation per head with the
           ones column producing l = sum(exp) in row 64.
  stage E: per (batch, q-chunk): r = 1/l (Ln+Exp), broadcast via PE outer
           product, normalize ctx^T to bf16, DMA into the AllToAll send
           buffer laid out as [dst-rank, 128 dims, 256 tokens].
           Per batch: AllToAll reshards ctx^T so each core holds ALL 1024
           ctx dims for its 256 tokens (~0.5 MB wire per rank — 8x less
           than gathering full ctx on every core).
  stage F: out^T[all od, my tokens] = Wo^T-tiles @ resharded ctx^T + bo.
           F(b0) runs while batch 1's AllToAll is in flight.

Host: passes x pre-transposed in bf16, wq/wk/wv column slices, FULL Wo in
bf16; reassembles from per-core token slices.
"""

import sys

for p in ("/opt/trn_rl_repo", "/root/.axon_site/_ro/trn_rl_repo"):
    if p not in sys.path:
        sys.path.insert(0, p)

import numpy as np
import ml_dtypes

import bass_rust
import concourse.bass as bass
import concourse.mybir as mybir
from concourse.bass_utils import run_bass_kernel_spmd
from concourse.masks import make_identity
from concourse.tile import TileContext

B, S, D = 2, 2048, 1024
H, DH = 16, 64
T = B * S              # 4096 tokens
NC = 8                 # cores
HG = D // NC           # 128 qkv dims per core (2 heads)
TC = S // NC           # 256 tokens per core per batch (stage F)
KT_D = D // 128        # 8 contraction tiles over d_model
NQC = S // 512         # 4 q-chunks per batch
INV_SCALE = 1.0 / float(np.sqrt(DH))  # 1/8
F32 = mybir.dt.float32
F32R = mybir.dt.float32r
BF16 = mybir.dt.bfloat16
BFNP = ml_dtypes.bfloat16


def _split_waits(nc, max_waits=1):
    """This walrus build accepts one sync-wait per instruction; Tile sometimes
    emits more. Split extras into preceding NoOps on the same engine."""
    n = 0
    for f in nc.m.functions:
        for bb in f.blocks:
            out = []
            for inst in bb.instructions:
                si = getattr(inst, "sync_info", None)
                if si is not None and si.on_wait and len(si.on_wait) > max_waits:
                    waits = list(si.on_wait)
                    head, rest = waits[:-max_waits], waits[-max_waits:]
                    k = 0
                    while head:
                        chunk, head = head[:max_waits], head[max_waits:]
                        out.append(mybir.InstNoOp(
                            name=f"{inst.name}-wsplit-{k}", ins=[], outs=[],
                            engine=inst.engine,
                            sync_info=bass_rust.SyncInfo(on_wait=chunk, on_update=[]),
                        ))
                        k += 1
                    si.on_wait = rest
                    n += 1
                out.append(inst)
            bb.instructions = out
    return n


def build_module(repeat=1, stages="BCDEF", do_collective=True):
    nc = bass.Bass()

    xT = nc.dram_tensor("xT", [D, T], BF16, kind="ExternalInput")
    wq = nc.dram_tensor("wq", [D, HG], BF16, kind="ExternalInput")
    wk = nc.dram_tensor("wk", [D, HG], BF16, kind="ExternalInput")
    wv = nc.dram_tensor("wv", [D, HG], BF16, kind="ExternalInput")
    wo = nc.dram_tensor("wo", [D, D], BF16, kind="ExternalInput")
    bq = nc.dram_tensor("bq", [HG, 1], F32, kind="ExternalInput")
    bk = nc.dram_tensor("bk", [HG, 1], F32, kind="ExternalInput")
    bv = nc.dram_tensor("bv", [HG, 1], F32, kind="ExternalInput")
    bo = nc.dram_tensor("bo", [128, KT_D], F32, kind="ExternalInput")
    yT = nc.dram_tensor("yT", [D, B * TC], F32, kind="ExternalOutput")

    # AllToAll staging per batch: send block j = (my 128 dims, tokens of
    # rank j); receive block j = (rank j's 128 dims, my 256 tokens)
    a2i = [nc.dram_tensor(f"a2i{b}", [NC, HG, TC], BF16) for b in range(B)]
    a2o = [nc.dram_tensor(f"a2o{b}", [NC, HG, TC], BF16) for b in range(B)]

    with TileContext(nc) as tc:
        with tc.tile_pool(name="persist", bufs=1) as pp:
            # qkv weights as [128, kt, 128]; full Wo as [128, kt, 1024]
            # (contraction chunk is the partition dim). Spread the loads
            # across engine queues so the first x tiles aren't stuck
            # behind the 2MB Wo load on the sync queue.
            w_sb = {}
            for name, dram, eng in (("wq", wq, nc.sync), ("wk", wk, nc.scalar),
                                    ("wv", wv, nc.scalar)):
                t = pp.tile([128, KT_D, HG], BF16, name=f"{name}_sb", tag=f"{name}_sb")
                eng.dma_start(out=t[:], in_=dram[:].rearrange("(kt p) n -> p kt n", p=128))
                w_sb[name] = t
            wo_sb = pp.tile([128, KT_D, D], BF16, name="wo_sb", tag="wo_sb")
            nc.gpsimd.dma_start(out=wo_sb[:],
                                in_=wo[:].rearrange("(kt p) n -> p kt n", p=128))
            b_sb = {}
            for name, dram in (("bq", bq), ("bk", bk), ("bv", bv)):
                t = pp.tile([HG, 1], F32, name=f"{name}_sb", tag=f"{name}_sb")
                nc.sync.dma_start(out=t[:], in_=dram[:])
                b_sb[name] = t
            bo_sb = pp.tile([128, KT_D], F32, name="bo_sb", tag="bo_sb")
            nc.sync.dma_start(out=bo_sb[:], in_=bo[:])

            # identity built in f32 (gpsimd memset can't write bf16 reliably),
            # then DVE-copied (rounds) into the bf16 tile matmul needs
            ident_f = pp.tile([128, 128], F32, name="ident_f", tag="ident_f")
            make_identity(nc, ident_f[:])
            ident = pp.tile([128, 128], BF16, name="ident", tag="ident")
            nc.vector.tensor_copy(ident[:], ident_f[:])
            # multiplicative causal mask for a diagonal 128x128 tile in
            # scores^T: tri01[r, c] = 1 where r <= c (k <= q), else 0
            tri_f = pp.tile([128, 128], F32, name="tri_f", tag="tri_f")
            nc.gpsimd.memset(tri_f[:], 1.0)
            nc.gpsimd.affine_select(
                out=tri_f[:], in_=tri_f[:],
                compare_op=mybir.AluOpType.is_ge, fill=0.0,
                base=0, pattern=[[1, 128]], channel_multiplier=-1,
            )
            tri01 = pp.tile([128, 128], BF16, name="tri01", tag="tri01")
            nc.vector.tensor_copy(tri01[:], tri_f[:])
            # ones row for the r-broadcast outer product (f32r, full speed)
            ones_r = pp.tile([65, 128], F32R, name="ones_r", tag="ones_r")
            of = pp.tile([65, 128], F32, name="of", tag="of")
            nc.vector.memset(of[:], 1.0)
            nc.vector.tensor_copy(ones_r[:], of[:])
            ones128 = pp.tile([128, 64], F32, name="ones128", tag="ones128")
            nc.vector.memset(ones128[:], 1.0)

            # per-batch Q^T/K^T/V^T so batch 1's projection overlaps batch 0's
            # attention
            qkvT = {}
            for name in ("qT", "kT", "vT"):
                qkvT[name] = [pp.tile([128, S], BF16, name=f"{name}{b}", tag=f"{name}{b}")
                              for b in range(B)]

            vaug = pp.tile([128, B * 2, S // 128, DH + 1], BF16, name="vaug", tag="vaug")
            nc.vector.tensor_copy(vaug[:, :, :, DH:DH + 1], ones128[:, :])
            # [65 used partitions, pair, q]; row 64 = l
            ctxu = pp.tile([128, B * 2, S], F32, name="ctxu", tag="ctxu")

            for _rep in range(repeat):
                if "B" not in stages:
                    break
                # ---------------- stage B: QKV projections (both batches) ----------------
                with (
                    tc.tile_pool(name="xt_pool", bufs=12) as xt_pool,
                    tc.tile_pool(name="psB", bufs=2, space="PSUM") as psB_pool,
                    tc.tile_pool(name="psT", bufs=2, space="PSUM") as psT_pool,
                ):
                    for b in range(B):
                        for tq in range(2):
                            t0 = tq * 1024
                            xts = []
                            for kt in range(KT_D):
                                xt = xt_pool.tile([128, 1024], BF16, name="xt", tag="xt")
                                nc.sync.dma_start(
                                    out=xt[:],
                                    in_=xT[kt * 128:(kt + 1) * 128,
                                           b * S + t0: b * S + t0 + 1024])
                                xts.append(xt)
                            for c2 in range(2):
                                ps = [psB_pool.tile([128, 512], F32, name=f"psB{i}",
                                                    tag=f"psB{i}") for i in range(3)]
                                for kt in range(KT_D):
                                    for pi, wname in enumerate(("wq", "wk", "wv")):
                                        nc.tensor.matmul(
                                            ps[pi][:],
                                            w_sb[wname][:, kt, :],
                                            xts[kt][:, c2 * 512:(c2 + 1) * 512],
                                            start=(kt == 0), stop=(kt == KT_D - 1),
                                        )
                                for pi, (dname, bname) in enumerate(
                                        (("qT", "bq"), ("kT", "bk"), ("vT", "bv"))):
                                    nc.vector.tensor_scalar_add(
                                        out=qkvT[dname][b][:, t0 + c2 * 512:
                                                           t0 + (c2 + 1) * 512],
                                        in0=ps[pi][:],
                                        scalar1=b_sb[bname][:, 0:1],
                                    )
                        if "C" not in stages:
                            continue
                        # ---- stage C: V^T -> V_aug for batch b ----
                        for h in range(2):
                            pr = b * 2 + h
                            for g in range(2):  # groups of 8 ktiles
                                pst = psT_pool.tile([128, 512], BF16, name="pst", tag="pst")
                                for j in range(8):
                                    kt = g * 8 + j
                                    nc.tensor.transpose(
                                        out=pst[:, j * DH:(j + 1) * DH],
                                        in_=qkvT["vT"][b][h * DH:(h + 1) * DH,
                                                          kt * 128:(kt + 1) * 128],
                                        identity=ident[h * DH:(h + 1) * DH,
                                                       h * DH:(h + 1) * DH],
                                    )
                                nc.vector.tensor_copy(
                                    vaug[:, pr, g * 8:(g + 1) * 8, 0:DH],
                                    pst[:],
                                )
                if "D" not in stages:
                    continue
                # ------- stages D-F, pipelined per (batch, q-chunk) -------
                with (
                    tc.tile_pool(name="psS", bufs=2, space="PSUM") as psS_pool,
                    tc.tile_pool(name="psC", bufs=2, space="PSUM") as psC_pool,
                    tc.tile_pool(name="misc", bufs=2, space="PSUM") as misc_pool,
                    tc.tile_pool(name="exp_pool", bufs=4) as exp_pool,
                    tc.tile_pool(name="rpool", bufs=2) as rpool,
                    tc.tile_pool(name="cn_pool", bufs=3) as cn_pool,
                    tc.tile_pool(name="gx_pool", bufs=2) as gx_pool,
                    tc.tile_pool(name="yo_pool", bufs=2) as yo_pool,
                ):
                    def stage_D(b):
                        pr0, pr1 = b * 2, b * 2 + 1
                        qT0 = qkvT["qT"][b][0:DH, :]
                        kT0 = qkvT["kT"][b][0:DH, :]
                        qT1 = qkvT["qT"][b][DH:2 * DH, :]
                        kT1 = qkvT["kT"][b][DH:2 * DH, :]
                        for qc in range(NQC):
                            q0 = qc * 512
                            n_kt = q0 // 128 + 4
                            # ---- stage D: both heads interleaved per k-tile ----
                            ps_c0 = psC_pool.tile([128, 512], F32, name="ps_c0",
                                                  tag="ps_ctx")
                            ps_c1 = psC_pool.tile([128, 512], F32, name="ps_c1",
                                                  tag="ps_ctx")
                            for kt in range(n_kt):
                                off = max(0, kt * 128 - q0)
                                ps_s = psS_pool.tile([128, 1024], F32, name="ps_s",
                                                     tag="ps_s")
                                # h0 on PE rows 0-63, h1 on rows 64-127: the two
                                # matmuls occupy different row groups and run
                                # concurrently
                                nc.tensor.matmul(
                                    ps_s[:, off:512],
                                    kT0[:, kt * 128:(kt + 1) * 128],
                                    qT0[:, q0 + off:q0 + 512],
                                    start=True, stop=True,
                                )
                                nc.tensor.matmul(
                                    ps_s[:, 512 + off:1024],
                                    kT1[:, kt * 128:(kt + 1) * 128],
                                    qT1[:, q0 + off:q0 + 512],
                                    start=True, stop=True,
                                )
                                ex = exp_pool.tile([128, 1024], BF16, name="ex", tag="ex")
                                # one exp over both heads' halves; the gap
                                # [512:512+off) holds stale-but-finite data
                                # that the ctx matmuls never read.
                                nc.scalar.activation(
                                    out=ex[:, off:1024], in_=ps_s[:, off:1024],
                                    func=mybir.ActivationFunctionType.Exp,
                                    scale=INV_SCALE,
                                )
                                if kt * 128 >= q0:
                                    # diagonal tile: multiplicative causal mask,
                                    # applied AFTER exp so DVE stays off the
                                    # PE->ACT critical path
                                    nc.vector.tensor_mul(
                                        out=ex[:, off:off + 128],
                                        in0=ex[:, off:off + 128],
                                        in1=tri01[:],
                                    )
                                    nc.vector.tensor_mul(
                                        out=ex[:, 512 + off:512 + off + 128],
                                        in0=ex[:, 512 + off:512 + off + 128],
                                        in1=tri01[:],
                                    )
                                nc.tensor.matmul(
                                    ps_c0[0:DH + 1, off:512],
                                    vaug[:, pr0, kt, :],
                                    ex[:, off:512],
                                    start=(kt == 0), stop=(kt == n_kt - 1),
                                    skip_group_check=True,
                                )
                                nc.tensor.matmul(
                                    ps_c1[0:DH + 1, off:512],
                                    vaug[:, pr1, kt, :],
                                    ex[:, 512 + off:1024],
                                    start=(kt == 0), stop=(kt == n_kt - 1),
                                    skip_group_check=True,
                                )
                            nc.vector.tensor_copy(
                                ctxu[0:DH + 1, pr0, q0:q0 + 512], ps_c0[0:DH + 1, :])
                            nc.vector.tensor_copy(
                                ctxu[0:DH + 1, pr1, q0:q0 + 512], ps_c1[0:DH + 1, :])
                            if "E" not in stages:
                                continue
                            # ---- stage E: r = 1/l = exp(-ln(l)), both heads ----
                            ln_f = rpool.tile([65, 2, 512], F32, name="ln_f", tag="ln_f")
                            nc.scalar.activation(
                                out=ln_f[64:65, :, :],
                                in_=ctxu[64:65, pr0:pr0 + 2, q0:q0 + 512],
                                func=mybir.ActivationFunctionType.Ln)
                            r_t = rpool.tile([65, 2, 512], F32R, name="r_t", tag="r_t")
                            nc.scalar.activation(
                                out=r_t[64:65, :, :], in_=ln_f[64:65, :, :],
                                func=mybir.ActivationFunctionType.Exp, scale=-1.0)
                            # normalize ctx^T to bf16, stage into the A2A send
                            # buffer blocks for dst ranks 2qc, 2qc+1
                            cn = cn_pool.tile([128, 512], BF16, name="cn", tag="cn")
                            for h in range(2):
                                bcst = misc_pool.tile([128, 512], F32, name="bc",
                                                      tag="efps")
                                nc.tensor.matmul(
                                    bcst[0:DH, :],
                                    ones_r[64:65, 0:DH],
                                    r_t[64:65, h, :],
                                    start=True, stop=True,
                                )
                                nc.vector.tensor_mul(
                                    out=cn[h * DH:(h + 1) * DH, :],
                                    in0=ctxu[0:DH, b * 2 + h, q0:q0 + 512],
                                    in1=bcst[0:DH, :],
                                )
                            nc.sync.dma_start(
                                out=a2i[b][2 * qc, :, :], in_=cn[:, 0:TC])
                            nc.sync.dma_start(
                                out=a2i[b][2 * qc + 1, :, :], in_=cn[:, TC:2 * TC])
                        if do_collective:
                            nc.gpsimd.collective_compute(
                                "AllToAll",
                                mybir.AluOpType.bypass,
                                ins=[a2i[b][:]],
                                outs=[a2o[b][:]],
                                replica_groups=[list(range(NC))],
                            )

                    def stage_F(b):
                        # out^T[all od, my 256 tokens] from resharded ctx^T
                        gx = gx_pool.tile([128, KT_D, TC], BF16, name="gx", tag="gx")
                        nc.sync.dma_start(
                            out=gx[:],
                            in_=a2o[b][:].rearrange("kt p t -> p kt t"))
                        for ot in range(KT_D):
                            ps_o = misc_pool.tile([128, 512], F32, name="ps_o",
                                                  tag="efps")
                            for kt in range(KT_D):
                                nc.tensor.matmul(
                                    ps_o[:, 0:TC],
                                    wo_sb[:, kt, ot * 128:(ot + 1) * 128],
                                    gx[:, kt, :],
                                    start=(kt == 0), stop=(kt == KT_D - 1),
                                )
                            yo = yo_pool.tile([128, TC], F32, name="yo", tag="yo")
                            nc.vector.tensor_scalar_add(
                                out=yo[:], in0=ps_o[:, 0:TC],
                                scalar1=bo_sb[:, ot:ot + 1])
                            nc.sync.dma_start(
                                out=yT[ot * 128:(ot + 1) * 128, b * TC:(b + 1) * TC],
                                in_=yo[:])

                    stage_D(0)
                    stage_D(1)
                    if "F" in stages and "E" in stages and do_collective:
                        stage_F(0)   # overlaps batch 1's AllToAll
                        stage_F(1)

    _split_waits(nc)
    return nc


def kernel(x, mask, Wq, bq, Wk, bk, Wv, bv, Wo, bo, trace=False, repeat=1, _in_maps_only=False):
    x = np.asarray(x, dtype=np.float32).reshape(T, D)
    xT = np.ascontiguousarray(x.T.astype(BFNP))
    wo_full = np.ascontiguousarray(np.asarray(Wo, np.float32).astype(BFNP))
    bo_full = np.ascontiguousarray(
        np.asarray(bo, np.float32).reshape(KT_D, 128).T)
    in_maps = []
    for c in range(NC):
        sl = slice(c * HG, (c + 1) * HG)
        in_maps.append({
            "xT": xT,
            "wq": np.ascontiguousarray(np.asarray(Wq, np.float32)[:, sl].astype(BFNP)),
            "wk": np.ascontiguousarray(np.asarray(Wk, np.float32)[:, sl].astype(BFNP)),
            "wv": np.ascontiguousarray(np.asarray(Wv, np.float32)[:, sl].astype(BFNP)),
            "wo": wo_full,
            "bq": np.ascontiguousarray(np.asarray(bq, np.float32)[sl].reshape(HG, 1)),
            "bk": np.ascontiguousarray(np.asarray(bk, np.float32)[sl].reshape(HG, 1)),
            "bv": np.ascontiguousarray(np.asarray(bv, np.float32)[sl].reshape(HG, 1)),
            "bo": bo_full,
        })
    if _in_maps_only:
        return in_maps
    nc = build_module(repeat=repeat)
    res = run_bass_kernel_spmd(nc, in_maps, core_ids=list(range(NC)), trace=trace)
    out = np.empty((B, S, D), dtype=np.float32)
    for c in range(NC):
        yt = res.results[c]["yT"]  # [D, B*TC]
        for b in range(B):
            out[b, c * TC:(c + 1) * TC, :] = yt[:, b * TC:(b + 1) * TC].T
    if trace:
        kernel.last_results = res
    return out
